# revision 1
# baseline (speedup 1.0000x reference)
"""Trainium2 Bass kernel for the BitwiseAutoencoder problem.

Pipeline (per core, data-parallel over batch: 8 of 64 batches per core):
  1. conv1d(1->256, k=256, stride=16, pad=256) as full-utilization matmuls
     against a stride-replicated frame matrix R built on-chip.
  2. relu + per-channel scale/bias fused into PSUM eviction; batchnorm
     statistics via bn_stats/bn_aggr, all-reduced across the 8 cores.
  3. BN affine folded into the transposed-conv weights (a*W2) and a per-phase
     bias vector (from d = beta - a*mu).
  4. convT(256->1, k=256, stride=16) as full-utilization matmuls producing
     per-tap partials, folded 16->1 via a DMA scatter + vector reduction.

The kernel is self-contained: shapes/sharding are hardcoded for
x: [64, 1, 32768] f32 and 8 NeuronCores.
"""

import numpy as np

import concourse.bass as bass
from concourse import bacc, mybir, tile
from concourse.bass_utils import run_bass_kernel_spmd

N_CORES = 8
B_FULL = 64
BPC = B_FULL // N_CORES  # 8 batches per core
T = 32768
K = 256
S = 16
BN_EPS = 1e-5

XP = T + 2 * K  # padded x length per batch (33280)
L = (T + 2 * K - K) // S + 1  # conv output length (2065)
RW = 2073  # R width: l in [0, 2064+8]
PW = XP // S  # 2080 phase columns

# conv free-dim tiles over L; EQUAL-WIDTH (they double as bn_stats groups and
# bn_aggr weights groups equally); 2065 = 5 * 413
CONV_TILES = [(413 * i, 413) for i in range(5)]

# deconv output tiles over w in [16, 2064); OF2 built in <=504-wide PSUM
# strips; 2048 = 683 + 683 + 682
WT = 683
U_TILES = [(16, 683), (699, 683), (1382, 682)]

F32 = mybir.dt.float32
BF16 = mybir.dt.bfloat16
AF = mybir.ActivationFunctionType


def _bf_split(a):
    """Exact hi/lo bf16 split: a == hi + lo to ~2^-17 relative."""
    import ml_dtypes
    hi = a.astype(ml_dtypes.bfloat16)
    lo = (a.astype(np.float64) - hi.astype(np.float64)).astype(ml_dtypes.bfloat16)
    return hi, lo


def _build():
    nc = bacc.Bacc("TRN2", target_bir_lowering=False, debug=False)

    # ---- external I/O ----
    # x in phase layout: x_ph[b, p, n] = x_pad[b, 16n + p]; bf16 hi/lo split
    xph_hi_t = nc.dram_tensor("x_ph_hi", [BPC, 16, PW], BF16, kind="ExternalInput")
    xph_lo_t = nc.dram_tensor("x_ph_lo", [BPC, 16, PW], BF16, kind="ExternalInput")
    w1t_hi_t = nc.dram_tensor("w1t_hi", [K, K], BF16, kind="ExternalInput")
    w1t_lo_t = nc.dram_tensor("w1t_lo", [K, K], BF16, kind="ExternalInput")
    bias1_t = nc.dram_tensor("bias1", [K], F32, kind="ExternalInput")
    w2_t = nc.dram_tensor("w2", [K, K], F32, kind="ExternalInput")  # [ch k, tap j]
    w2fold_t = nc.dram_tensor("w2fold", [K, 16], F32, kind="ExternalInput")
    gamma_t = nc.dram_tensor("gamma", [K], F32, kind="ExternalInput")
    beta_t = nc.dram_tensor("beta", [K], F32, kind="ExternalInput")
    cb16_t = nc.dram_tensor("cb16", [16], F32, kind="ExternalInput")
    y_t = nc.dram_tensor("y", [BPC, T], F32, kind="ExternalOutput")

    with tile.TileContext(nc) as tc:
        with (
            tc.tile_pool(name="persist", bufs=1) as persist,
            tc.tile_pool(name="rpool", bufs=2) as rpool,
            tc.tile_pool(name="hevt", bufs=2) as hevt,
            tc.tile_pool(name="of2pool", bufs=2) as of2pool,
            tc.tile_pool(name="t4pool", bufs=1) as t4pool,
            tc.tile_pool(name="yacc", bufs=2) as yaccpool,
            tc.tile_pool(name="smalls", bufs=1) as smalls,
            tc.tile_pool(name="psum_conv", bufs=3, space="PSUM") as psum_conv,
            tc.tile_pool(name="psum_j0", bufs=4, space="PSUM") as psum_j0,
            tc.tile_pool(name="psum_cp", bufs=1, space="PSUM") as psum_cp,
            tc.tile_pool(name="dram", bufs=1, space="DRAM") as dram,
        ):
            # ---- load weights/constants into SBUF ----
            w1t_hi_sb, w1t_lo_sb = [], []
            for h in range(2):
                wh = persist.tile([128, K], BF16, tag=f"w1th{h}", name=f"w1th{h}")
                nc.scalar.dma_start(out=wh[:], in_=w1t_hi_t[128 * h:128 * (h + 1), :])
                w1t_hi_sb.append(wh)
                wl = persist.tile([128, K], BF16, tag=f"w1tl{h}", name=f"w1tl{h}")
                nc.scalar.dma_start(out=wl[:], in_=w1t_lo_t[128 * h:128 * (h + 1), :])
                w1t_lo_sb.append(wl)
            w2_sb = []  # per ch-half kc: [128, 256] (rows: ch k-128kc, cols: tap j)
            w2fold_sb = []
            for kc in range(2):
                wt = persist.tile([128, K], F32, tag=f"w2{kc}", name=f"w2{kc}")
                nc.scalar.dma_start(out=wt[:], in_=w2_t[128 * kc:128 * (kc + 1), :])
                w2_sb.append(wt)
                wf = persist.tile([128, 16], F32, tag=f"w2fold{kc}", name=f"w2fold{kc}")
                nc.scalar.dma_start(out=wf[:], in_=w2fold_t[128 * kc:128 * (kc + 1), :])
                w2fold_sb.append(wf)
            bias1_sb, gamma_sb, beta_sb = [], [], []
            for cc in range(2):
                for lst, src in ((bias1_sb, bias1_t), (gamma_sb, gamma_t), (beta_sb, beta_t)):
                    tl = persist.tile([128, 1], F32, tag=f"v{cc}_{id(src) % 997}", name=f"vec{cc}_{id(src) % 997}")
                    nc.scalar.dma_start(out=tl[:], in_=src[128 * cc:128 * (cc + 1)])
                    lst.append(tl)
            cb_sb = persist.tile([16, 1], F32, tag="cb")
            nc.scalar.dma_start(out=cb_sb[:], in_=cb16_t[:])
            eps_sb = persist.tile([128, 1], F32, tag="eps")
            nc.vector.memset(eps_sb[:], BN_EPS)

            # H: conv output (post-relu), kept in SBUF as an exact bf16
            # hi/lo pair (same bytes as f32, enables 1-cycle/row matmuls).
            Hh = [persist.tile([128, BPC, L], BF16, tag=f"Hh{cc}", name=f"Hh{cc}") for cc in range(2)]
            Hl = [persist.tile([128, BPC, L], BF16, tag=f"Hl{cc}", name=f"Hl{cc}") for cc in range(2)]
            # bn_stats accumulator: per cc: 8 batches x 5 equal groups
            stats = [persist.tile([128, 5 * BPC, 6], F32, tag=f"st{cc}", name=f"st{cc}") for cc in range(2)]

            # ================= phase 1: conv + stats =================
            for b in range(BPC):
                # R[16g+p, l] = x_pad[16(l+g) + p] = x_ph[b, p, l+g]
                # one DMA each for the hi/lo halves (host pre-split)
                Rh = rpool.tile([128, RW], BF16, tag="Rh", name=f"Rh{b}")
                Rl = rpool.tile([128, RW], BF16, tag="Rl", name=f"Rl{b}")
                nc.sync.dma_start(
                    out=Rh[:],
                    in_=bass.AP(tensor=xph_hi_t, offset=b * XP,
                                ap=[[1, 8], [PW, 16], [1, RW]]),
                )
                nc.sync.dma_start(
                    out=Rl[:],
                    in_=bass.AP(tensor=xph_lo_t, offset=b * XP,
                                ap=[[1, 8], [PW, 16], [1, RW]]),
                )
                for cc in range(2):
                    for gi, (l0, w) in enumerate(CONV_TILES):
                        ps = psum_conv.tile([128, 416], F32, tag="pconv")
                        cs = slice(128 * cc, 128 * (cc + 1))
                        first = True
                        for h in range(2):
                            for lhsT, rhs in (
                                (w1t_hi_sb[h], Rh), (w1t_hi_sb[h], Rl),
                                (w1t_lo_sb[h], Rh),
                            ):
                                nc.tensor.matmul(
                                    ps[:, :w], lhsT[:, cs],
                                    rhs[:, l0 + 8 * h:l0 + 8 * h + w],
                                    start=first, stop=(h == 1 and lhsT is w1t_lo_sb[1]),
                                )
                                first = False
                        # h = relu(psum + bias); conv_scale folded into W on host
                        hv = hevt.tile([128, 416], F32, tag="hevt")
                        nc.scalar.activation(
                            out=hv[:, :w], in_=ps[:, :w], func=AF.Relu,
                            bias=bias1_sb[cc][:, 0:1], scale=1.0,
                        )
                        nc.vector.bn_stats(
                            out=stats[cc][:, 5 * b + gi, :], in_=hv[:, :w],
                        )
                        # exact bf16 hi/lo split of h (on the otherwise
                        # idle GPSIMD engine)
                        nc.gpsimd.tensor_copy(Hh[cc][:, b, l0:l0 + w], hv[:, :w])
                        nc.gpsimd.tensor_sub(
                            Hl[cc][:, b, l0:l0 + w], hv[:, :w],
                            Hh[cc][:, b, l0:l0 + w],
                        )

            # ================= phase 2: global BN stats =================
            bounce_in = dram.tile([2, 128, 2], F32)
            bounce_out = dram.tile([N_CORES, 2, 128, 2], F32)
            for cc in range(2):
                mv = smalls.tile([128, 2], F32, tag=f"mv{cc}", name=f"mv{cc}")
                nc.vector.bn_aggr(out=mv[:], in_=stats[cc][:])
                # pack [mean, E[h^2]] = [mean, var + mean^2]
                pk = smalls.tile([128, 2], F32, tag=f"pk{cc}", name=f"pk{cc}")
                nc.vector.tensor_mul(pk[:, 0:1], mv[:, 0:1], mv[:, 0:1])
                nc.vector.tensor_add(pk[:, 1:2], mv[:, 1:2], pk[:, 0:1])
                nc.vector.tensor_copy(pk[:, 0:1], mv[:, 0:1])
                nc.sync.dma_start(out=bounce_in[cc, :, :], in_=pk[:])
            # AllGather (cheaper than AllReduce) + local sum over cores
            nc.gpsimd.collective_compute(
                "AllGather",
                mybir.AluOpType.bypass,
                replica_groups=[list(range(N_CORES))],
                ins=[bounce_in.opt()],
                outs=[bounce_out.opt()],
            )
            a_sb, d_sb = [], []
            for cc in range(2):
                # gathered[core, cc, p, v] -> sbuf [128, 2, 8] (v, core)
                gall = smalls.tile([128, 2, N_CORES], F32, tag=f"gall{cc}", name=f"gall{cc}")
                nc.sync.dma_start(
                    out=gall[:],
                    in_=bass.AP(tensor=bounce_out.tensor,
                                offset=bounce_out.offset + cc * 256,
                                ap=[[2, 128], [1, 2], [512, N_CORES]]),
                )
                gst = smalls.tile([128, 2], F32, tag=f"gst{cc}", name=f"gst{cc}")
                nc.vector.reduce_sum(gst[:], gall[:], axis=mybir.AxisListType.X)
                # gmean = sum/8 ; gE2 = sum/8 ; gvar = gE2 - gmean^2
                gm = smalls.tile([128, 2], F32, tag=f"gm{cc}", name=f"gm{cc}")
                nc.vector.tensor_scalar_mul(gm[:], gst[:], 1.0 / N_CORES)
                gvar = smalls.tile([128, 1], F32, tag=f"gvar{cc}", name=f"gvar{cc}")
                nc.vector.tensor_mul(gvar[:], gm[:, 0:1], gm[:, 0:1])
                nc.vector.tensor_sub(gvar[:], gm[:, 1:2], gvar[:])
                sd = smalls.tile([128, 1], F32, tag=f"sd{cc}", name=f"sd{cc}")
                nc.scalar.activation(out=sd[:], in_=gvar[:], func=AF.Sqrt,
                                     bias=eps_sb[:, 0:1], scale=1.0)
                rinv = smalls.tile([128, 1], F32, tag=f"rinv{cc}", name=f"rinv{cc}")
                nc.vector.reciprocal(rinv[:], sd[:])
                a = smalls.tile([128, 1], F32, tag=f"a{cc}", name=f"a{cc}")
                nc.vector.tensor_mul(a[:], rinv[:], gamma_sb[cc][:])
                # d = beta - a * gmean
                d = smalls.tile([128, 1], F32, tag=f"d{cc}", name=f"d{cc}")
                nc.vector.tensor_mul(d[:], a[:], gm[:, 0:1])
                nc.vector.tensor_sub(d[:], beta_sb[cc][:], d[:])
                a_sb.append(a)
                d_sb.append(d)
            # fold BN scale into deconv weights (in place), then bf16-split
            w2a_hi, w2a_lo = [], []
            for kc in range(2):
                nc.vector.tensor_scalar_mul(w2_sb[kc][:], w2_sb[kc][:], a_sb[kc][:, 0:1])
                wh = persist.tile([128, K], BF16, tag=f"w2ah{kc}", name=f"w2ah{kc}")
                wl = persist.tile([128, K], BF16, tag=f"w2al{kc}", name=f"w2al{kc}")
                nc.vector.tensor_copy(wh[:], w2_sb[kc][:])
                nc.vector.tensor_sub(wl[:], w2_sb[kc][:], wh[:])
                w2a_hi.append(wh)
                w2a_lo.append(wl)
            # per-phase bias: CP[p] = sum_k w2fold[k, p] d[k] + ct_scale*ct_b
            pcp = psum_cp.tile([16, 1], F32, tag="pcp")
            nc.tensor.matmul(pcp[:], w2fold_sb[0][:], d_sb[0][:], start=True, stop=False)
            nc.tensor.matmul(pcp[:], w2fold_sb[1][:], d_sb[1][:], start=False, stop=True)
            cp16 = smalls.tile([16, 1], F32, tag="cp16")
            nc.vector.tensor_add(cp16[:], pcp[:], cb_sb[:])
            cp_dram = dram.tile([16], F32)
            nc.sync.dma_start(out=cp_dram[:], in_=cp16[:])
            cpb = smalls.tile([128, 1], F32, tag="cpb")
            nc.sync.dma_start(
                out=cpb[:],
                in_=bass.AP(tensor=cp_dram.tensor, offset=cp_dram.offset,
                            ap=[[0, 8], [1, 16], [0, 1]]),
            )

            # ================= phase 3: deconv =================
            for (w0, wt) in U_TILES:
                w7 = wt + 7
                t4a = t4pool.tile([128, 4, WT], F32, tag="T4A", name=f"t4a_{w0}")
                t4b = t4pool.tile([128, 4, WT], F32, tag="T4B", name=f"t4b_{w0}")
                for b in range(BPC):
                    # all 12 matmuls accumulate into one PSUM tile; the
                    # tap-half fold OF2[r, n] = OF[r, n] + OF[r+128, n-8] is
                    # realized by shifting the rhs slice for the j>=128 taps.
                    of2 = of2pool.tile([128, WT + 7], F32, tag="OF2", name=f"of2_{w0}_{b}")
                    for s0 in range(0, w7, 504):
                        sw = min(504, w7 - s0)
                        ps = psum_j0.tile([128, 504], F32, tag="pj0")
                        nmm = 0
                        for th, off in ((0, 7), (128, 15)):
                            for kc in range(2):
                                js = slice(th, th + 128)
                                for lhsT, rhs in (
                                    (w2a_hi[kc], Hh[kc]), (w2a_hi[kc], Hl[kc]),
                                    (w2a_lo[kc], Hh[kc]),
                                ):
                                    nc.tensor.matmul(
                                        ps[:, :sw], lhsT[:, js],
                                        rhs[:, b, w0 - off + s0:w0 - off + s0 + sw],
                                        start=(nmm == 0), stop=(nmm == 11),
                                    )
                                    nmm += 1
                        nc.vector.tensor_copy(of2[:, s0:s0 + sw], ps[:, :sw])
                    # scatter the 8 m-groups into (batch, phase)-stacked
                    # slots; alternate HWDGE (sync) / SWDGE (gpsimd) queues
                    for m in range(8):
                        eng = nc.sync if ((b + m) % 2 == 0) else nc.gpsimd
                        t4 = t4a if m < 4 else t4b
                        eng.dma_start(
                            out=t4[16 * b:16 * (b + 1), m % 4, :wt],
                            in_=of2[16 * m:16 * (m + 1), 7 - m:7 - m + wt],
                        )
                # reduce over m and add the per-phase bias; done in two
                # partition halves so batches 0-3 retire while 4-7 scatter
                ya = yaccpool.tile([128, WT], F32, tag="ya")
                for hb in range(2):
                    rows = slice(64 * hb, 64 * (hb + 1))
                    nc.vector.tensor_add(ya[rows, :wt], t4a[rows, 0, :wt],
                                         t4a[rows, 1, :wt])
                    for m in range(2, 4):
                        nc.vector.tensor_add(ya[rows, :wt], ya[rows, :wt],
                                             t4a[rows, m, :wt])
                    for m in range(4):
                        nc.vector.tensor_add(ya[rows, :wt], ya[rows, :wt],
                                             t4b[rows, m, :wt])
                    nc.vector.tensor_scalar_add(ya[rows, :wt], ya[rows, :wt],
                                                cpb[rows, 0:1])
                    for b in range(4 * hb, 4 * (hb + 1)):
                        nc.scalar.dma_start(
                            out=bass.AP(tensor=y_t, offset=b * T + 16 * (w0 - 16),
                                        ap=[[1, 16], [16, wt]]),
                            in_=ya[16 * b:16 * (b + 1), :wt],
                        )
    nc.compile()
    return nc


_NC_CACHE = None


def _get_nc():
    global _NC_CACHE
    if _NC_CACHE is None:
        _NC_CACHE = _build()
    return _NC_CACHE


def _host_prep(inputs):
    conv_w = np.asarray(inputs["conv_w"], dtype=np.float32)
    conv_b = np.asarray(inputs["conv_b"], dtype=np.float32)
    conv_gate = np.asarray(inputs["conv_gate"], dtype=np.float32)
    conv_scale = np.asarray(inputs["conv_scale"], dtype=np.float32)
    bn_gamma = np.asarray(inputs["bn_gamma"], dtype=np.float32)
    bn_beta = np.asarray(inputs["bn_beta"], dtype=np.float32)
    ct_w = np.asarray(inputs["ct_w"], dtype=np.float32)
    ct_b = np.asarray(inputs["ct_b"], dtype=np.float32)
    ct_gate = np.asarray(inputs["ct_gate"], dtype=np.float32)
    ct_scale = np.asarray(inputs["ct_scale"], dtype=np.float32)

    W1 = conv_w[:, 0, :] * (conv_gate[:, 0, :] + 1.0) * 0.5  # [c, j]
    W1 = W1 * conv_scale[:, None]
    bias1 = conv_scale * conv_b
    w1t = np.ascontiguousarray(W1.T)  # [j, c]
    w1t_hi, w1t_lo = _bf_split(w1t)

    W2 = ct_w[:, 0, :] * (ct_gate[:, 0, :] + 1.0) * 0.5  # [k, j]
    W2 = W2 * float(ct_scale[0])
    w2fold = np.ascontiguousarray(W2.reshape(K, 16, 16).sum(axis=1))  # [k, p]
    cb16 = np.full(16, float(ct_scale[0]) * float(ct_b[0]), dtype=np.float32)

    return {
        "w1t_hi": np.ascontiguousarray(w1t_hi),
        "w1t_lo": np.ascontiguousarray(w1t_lo),
        "bias1": bias1.astype(np.float32),
        "w2": np.ascontiguousarray(W2).astype(np.float32),
        "w2fold": w2fold.astype(np.float32),
        "gamma": bn_gamma.astype(np.float32),
        "beta": bn_beta.astype(np.float32),
        "cb16": cb16,
    }


def kernel(**inputs) -> np.ndarray:
    x = np.asarray(inputs["x"], dtype=np.float32)  # [64, 1, 32768]
    shared = _host_prep(inputs)
    nc = _get_nc()

    in_maps = []
    for c in range(N_CORES):
        shard = x[BPC * c:BPC * (c + 1), 0, :]  # [8, T]
        xpad = np.zeros((BPC, XP), dtype=np.float32)
        xpad[:, K:K + T] = shard
        # phase layout: x_ph[b, p, n] = x_pad[b, 16n + p], bf16 hi/lo split
        xph = np.ascontiguousarray(xpad.reshape(BPC, PW, 16).transpose(0, 2, 1))
        xph_hi, xph_lo = _bf_split(xph)
        m = dict(shared)
        m["x_ph_hi"] = np.ascontiguousarray(xph_hi)
        m["x_ph_lo"] = np.ascontiguousarray(xph_lo)
        in_maps.append(m)

    res = run_bass_kernel_spmd(nc, in_maps, core_ids=list(range(N_CORES)))
    y = np.concatenate([res.results[c]["y"].reshape(BPC, 1, T) for c in range(N_CORES)], axis=0)
    return y.astype(np.float32)



# revision 4
# speedup vs baseline: 2.7344x; 2.7344x over previous
"""Trainium2 Bass kernel for the BitwiseAutoencoder problem.

Pipeline (per core, data-parallel over batch: 8 of 64 batches per core):
  1. conv1d(1->256, k=256, stride=16, pad=256) as bf16 matmuls against a
     stride-replicated frame matrix R (one gather DMA per batch, resident).
  2. PSUM eviction fuses relu + bias on the Activation engine (4-bank-wide
     ops), writing H directly as bf16; the eviction's accum_out produces
     sum(h) for free.  sum(h^2) comes from DVE bn_stats on most units plus
     ACT Square-with-accum on two groups (engine balance).
  3. [Sh, Sh2] all-gathered across the 8 cores; BN affine folded into the
     transposed-conv weights (a*W2, bf16) and a per-phase bias vector.
  4. convT(256->1, k=256, stride=16) as bf16 matmuls; tap-half fold done in
     PSUM via shifted rhs; the remaining 8 tap groups are regrouped through
     a DRAM bounce (partition-restructuring DMAs) and folded with a bf16
     add tree on DVE.  Output is written phase-major (contiguous DMA) and
     transposed on the host.

Self-contained: shapes/sharding hardcoded for x: [64, 1, 32768] f32, 8 cores.
"""

import numpy as np

import concourse.bass as bass
from concourse import bacc, mybir, tile
from concourse.bass_utils import run_bass_kernel_spmd

N_CORES = 8
B_FULL = 64
BPC = B_FULL // N_CORES  # 8 batches per core
T = 32768
K = 256
S = 16
BN_EPS = 1e-5

XP = T + 2 * K  # padded x length per batch (33280)
L = (T + 2 * K - K) // S + 1  # conv output length (2065)
RW = 2073  # R width: l in [0, 2064+8]
PW = XP // S  # 2080 phase columns

UW = 413   # conv matmul unit width (L = 5*413)
GW = 4 * UW  # eviction group width (1652); 10 groups per cc half
NG = 20
NU = 80

# deconv output tiles over u' in [16, 2064); 2048 = 683 + 683 + 682
U_TILES = [(16, 683), (699, 683), (1382, 682)]
OFW = 690  # of2 free width (wt + 7)

F32 = mybir.dt.float32
BF16 = mybir.dt.bfloat16
AF = mybir.ActivationFunctionType
ALU = mybir.AluOpType

# groups whose sum(h^2) is computed by an ACT Square pass instead of DVE
# bn_stats (one per cc half, for ACT/DVE balance)
ACT_SQ_GROUPS = (0, 10)


def _flat_ap(tl, n0, dims):
    """Raw AP over an SBUF tile at flat free-offset n0 with given free dims."""
    full = tl[:]
    return bass.AP(tensor=full.tensor, offset=full.offset + n0,
                   ap=[[full.ap[0][0], 128]] + dims)


def _build():
    nc = bacc.Bacc("TRN2", target_bir_lowering=False, debug=False)

    # ---- external I/O ----
    xph_t = nc.dram_tensor("x_ph", [BPC, 16, PW], BF16, kind="ExternalInput")
    w1t_t = nc.dram_tensor("w1t", [128, 2, K], BF16, kind="ExternalInput")
    vecs_t = nc.dram_tensor("vecs", [128, 2, 3], F32, kind="ExternalInput")
    w2_t = nc.dram_tensor("w2", [128, 2, K], F32, kind="ExternalInput")
    w2fold_t = nc.dram_tensor("w2fold", [128, 2, 16], F32, kind="ExternalInput")
    cb16_t = nc.dram_tensor("cb16", [16], F32, kind="ExternalInput")
    y_t = nc.dram_tensor("y", [BPC, 16, 2048], F32, kind="ExternalOutput")

    with tile.TileContext(nc) as tc:
        with (
            tc.tile_pool(name="persist", bufs=1) as persist,
            tc.tile_pool(name="sqpool", bufs=1) as sqpool,
            tc.tile_pool(name="of2pool", bufs=3) as of2pool,
            tc.tile_pool(name="t4pool", bufs=2) as t4pool,
            tc.tile_pool(name="etpool", bufs=2) as etpool,
            tc.tile_pool(name="yacc", bufs=2) as yaccpool,
            tc.tile_pool(name="smalls", bufs=1) as smalls,
            tc.tile_pool(name="dram", bufs=1, space="DRAM") as dram,
        ):
            # ---- load weights/constants ----
            w1t_sb = persist.tile([128, 2, K], BF16, tag="w1t")
            nc.sync.dma_start(out=w1t_sb[:], in_=w1t_t[:, :, :])
            vecs_sb = persist.tile([128, 2, 3], F32, tag="vecs")
            nc.sync.dma_start(out=vecs_sb[:], in_=vecs_t[:, :, :])
            w2_sb = persist.tile([128, 2, K], F32, tag="w2")
            nc.sync.dma_start(out=w2_sb[:], in_=w2_t[:, :, :])
            w2fold_sb = persist.tile([128, 2, 16], F32, tag="w2fold")
            nc.sync.dma_start(out=w2fold_sb[:], in_=w2fold_t[:, :, :])
            cb_sb = persist.tile([16, 1], F32, tag="cb")
            nc.sync.dma_start(out=cb_sb[:], in_=cb16_t[:])
            eps_sb = persist.tile([128, 1], F32, tag="eps")
            nc.vector.memset(eps_sb[:], BN_EPS)

            # R frame matrices, one per batch, all resident
            R = []
            for b in range(BPC):
                r = persist.tile([128, RW], BF16, tag=f"R{b}", name=f"R{b}")
                nc.sync.dma_start(
                    out=r[:],
                    in_=bass.AP(tensor=xph_t, offset=b * XP,
                                ap=[[1, 8], [PW, 16], [1, RW]]),
                )
                R.append(r)

            # H: conv output (post-relu) bf16, flat layout (cc, b, l)
            H = persist.tile([128, 2, BPC, L], BF16, tag="H", name="H")
            sums1 = persist.tile([128, NG], F32, tag="s1", name="s1")
            sums2a = persist.tile([128, 2], F32, tag="s2a", name="s2a")
            stats = persist.tile([128, NU, 6], F32, tag="st", name="st")
            sq = sqpool.tile([128, GW], BF16, tag="sq", name="sq")

            # ================= phase 1: conv + stats =================
            with tc.tile_pool(name="psum_conv", bufs=2, space="PSUM") as pconv:
                for g in range(NG):
                    cc = g // 10
                    q = g % 10
                    ps = pconv.tile([128, 4, 512], F32, tag="pc")
                    for i in range(4):
                        w = 4 * q + i  # within-cc unit: 5*b + gi
                        b, gi = w // 5, w % 5
                        l0 = UW * gi
                        for h in range(2):
                            nc.tensor.matmul(
                                ps[:, i, 0:UW],
                                w1t_sb[:, h, 128 * cc:128 * (cc + 1)],
                                R[b][:, l0 + 8 * h:l0 + 8 * h + UW],
                                start=(h == 0), stop=(h == 1),
                            )
                    n0 = GW * q + 16520 * cc
                    out_ap = _flat_ap(H, n0, [[UW, 4], [1, UW]])
                    # relu+bias eviction -> bf16 H; accum gives sum(h)
                    nc.scalar.activation(
                        out=out_ap, in_=ps[:, :, 0:UW], func=AF.Relu,
                        bias=vecs_sb[:, cc, 0:1], scale=1.0,
                        accum_out=sums1[:, g:g + 1],
                    )
                    if g in ACT_SQ_GROUPS:
                        # sum(h^2) for this group on ACT
                        nc.scalar.activation(
                            out=sq[:, :], in_=_flat_ap(H, n0, [[1, GW]]),
                            func=AF.Square,
                            accum_out=sums2a[:, cc:cc + 1],
                        )
                    else:
                        for i in range(4):
                            u = 4 * g + i
                            nc.vector.bn_stats(
                                out=stats[:, u, :],
                                in_=_flat_ap(H, 413 * u, [[1, UW]]),
                            )

            # ================= phase 2: global BN =================
            bounce_in = dram.tile([128, 4], F32)
            bounce_out = dram.tile([N_CORES, 128, 4], F32)
            pk = smalls.tile([128, 4], F32, tag="pk")
            n_rest = float(36 * UW)
            for cc in range(2):
                # total sum(h) for this half
                nc.vector.reduce_sum(pk[:, 2 * cc:2 * cc + 1],
                                     sums1[:, 10 * cc:10 * cc + 10],
                                     axis=mybir.AxisListType.X)
                # sum(h^2): ACT group + bn_stats units 4..39 (44..79)
                mv = smalls.tile([128, 2], F32, tag=f"mv{cc}", name=f"mv{cc}")
                nc.vector.bn_aggr(out=mv[:],
                                  in_=stats[:, 40 * cc + 4:40 * cc + 40, :])
                e2r = smalls.tile([128, 1], F32, tag=f"e2r{cc}", name=f"e2r{cc}")
                nc.vector.tensor_mul(e2r[:], mv[:, 0:1], mv[:, 0:1])
                nc.vector.tensor_add(e2r[:], e2r[:], mv[:, 1:2])
                nc.vector.tensor_scalar_mul(e2r[:], e2r[:], n_rest)
                nc.vector.tensor_add(pk[:, 2 * cc + 1:2 * cc + 2],
                                     e2r[:], sums2a[:, cc:cc + 1])
            nc.sync.dma_start(out=bounce_in[:, :], in_=pk[:])
            nc.gpsimd.collective_compute(
                "AllGather",
                mybir.AluOpType.bypass,
                replica_groups=[list(range(N_CORES))],
                ins=[bounce_in.opt()],
                outs=[bounce_out.opt()],
            )
            gall = smalls.tile([128, 4, N_CORES], F32, tag="gall")
            nc.sync.dma_start(
                out=gall[:],
                in_=bass.AP(tensor=bounce_out.tensor, offset=bounce_out.offset,
                            ap=[[4, 128], [1, 4], [512, N_CORES]]),
            )
            gsum = smalls.tile([128, 4], F32, tag="gsum")
            nc.vector.reduce_sum(gsum[:], gall[:], axis=mybir.AxisListType.X)
            inv_n = 1.0 / (N_CORES * BPC * L)
            a_sb, d_sb = [], []
            for cc in range(2):
                mE = smalls.tile([128, 2], F32, tag=f"mE{cc}", name=f"mE{cc}")
                nc.vector.tensor_scalar_mul(mE[:], gsum[:, 2 * cc:2 * cc + 2], inv_n)
                gvar = smalls.tile([128, 1], F32, tag=f"gv{cc}", name=f"gv{cc}")
                nc.vector.tensor_mul(gvar[:], mE[:, 0:1], mE[:, 0:1])
                nc.vector.tensor_sub(gvar[:], mE[:, 1:2], gvar[:])
                sd = smalls.tile([128, 1], F32, tag=f"sd{cc}", name=f"sd{cc}")
                nc.scalar.activation(out=sd[:], in_=gvar[:], func=AF.Sqrt,
                                     bias=eps_sb[:, 0:1], scale=1.0)
                rinv = smalls.tile([128, 1], F32, tag=f"ri{cc}", name=f"ri{cc}")
                nc.vector.reciprocal(rinv[:], sd[:])
                a = smalls.tile([128, 1], F32, tag=f"a{cc}", name=f"a{cc}")
                nc.vector.tensor_mul(a[:], rinv[:], vecs_sb[:, cc, 1:2])
                d = smalls.tile([128, 1], F32, tag=f"d{cc}", name=f"d{cc}")
                nc.vector.tensor_mul(d[:], a[:], mE[:, 0:1])
                nc.vector.tensor_sub(d[:], vecs_sb[:, cc, 2:3], d[:])
                a_sb.append(a)
                d_sb.append(d)
            # fold BN scale into deconv weights -> bf16
            w2a = persist.tile([128, 2, K], BF16, tag="w2a", name="w2a")
            for kc in range(2):
                nc.vector.tensor_scalar_mul(w2_sb[:, kc, :], w2_sb[:, kc, :],
                                            a_sb[kc][:, 0:1])
                nc.vector.tensor_copy(w2a[:, kc, :], w2_sb[:, kc, :])

            with (
                tc.tile_pool(name="psum_cp", bufs=1, space="PSUM") as psum_cp,
                tc.tile_pool(name="psum_dec", bufs=4, space="PSUM") as pdec,
            ):
                # per-phase bias: cp[p] = sum_k w2fold[k, p] d[k] + ct_scale*ct_b
                pcp = psum_cp.tile([16, 1], F32, tag="pcp")
                nc.tensor.matmul(pcp[:], w2fold_sb[:, 0, :], d_sb[0][:],
                                 start=True, stop=False)
                nc.tensor.matmul(pcp[:], w2fold_sb[:, 1, :], d_sb[1][:],
                                 start=False, stop=True)
                cp16 = smalls.tile([16, 1], F32, tag="cp16")
                nc.vector.tensor_add(cp16[:], pcp[:], cb_sb[:])
                cp_dram = dram.tile([16], F32)
                nc.sync.dma_start(out=cp_dram[:], in_=cp16[:])
                # cpb[8p + b] = cp[p]
                cpb = smalls.tile([128, 1], F32, tag="cpb")
                nc.sync.dma_start(
                    out=cpb[:],
                    in_=bass.AP(tensor=cp_dram.tensor, offset=cp_dram.offset,
                                ap=[[1, 16], [0, 8], [0, 1]]),
                )

                # ================= phase 3: deconv =================
                of2d = [dram.tile([128, BPC, OFW], BF16, name=f"of2d{i}")
                        for i in range(2)]
                for ti, (w0, wt) in enumerate(U_TILES):
                    w7 = wt + 7
                    dbuf = of2d[ti % 2]
                    for b in range(BPC):
                        of2 = of2pool.tile([128, OFW], BF16, tag="OF2",
                                           name=f"of2_{w0}_{b}")
                        # tap-half fold in PSUM: OF2[r, n] = OF[r, n] + OF[r+128, n-8]
                        for s0 in range(0, w7, 504):
                            sw = min(504, w7 - s0)
                            ps = pdec.tile([128, 504], F32, tag="pd")
                            nmm = 0
                            for th, off in ((0, 7), (128, 15)):
                                for kc in range(2):
                                    nc.tensor.matmul(
                                        ps[:, :sw],
                                        w2a[:, kc, th:th + 128],
                                        H[:, kc, b, w0 - off + s0:w0 - off + s0 + sw],
                                        start=(nmm == 0), stop=(nmm == 3),
                                    )
                                    nmm += 1
                            nc.scalar.copy(of2[:, s0:s0 + sw], ps[:, :sw])
                        # bounce this batch's taps to DRAM (SWDGE, Pool seq)
                        nc.gpsimd.dma_start(out=dbuf[:, b, 0:w7],
                                            in_=of2[:, 0:w7])
                    # partition regroup via strided DRAM reads:
                    # t4[8p + b, m, u] = of2[16m + p, b, u + 7 - m]
                    t4 = t4pool.tile([128, 8, wt], BF16, tag="T4",
                                     name=f"t4_{w0}")
                    for m in range(8):
                        nc.sync.dma_start(
                            out=t4[:, m, :wt],
                            in_=bass.AP(
                                tensor=dbuf.tensor,
                                offset=dbuf.offset + (16 * m) * (BPC * OFW) + (7 - m),
                                ap=[[BPC * OFW, 16], [OFW, 8], [1, wt]]),
                        )
                    # fold the 8 tap groups: bf16 add tree, f32 root
                    et = [etpool.tile([128, wt], BF16, tag=f"e{i}",
                                      name=f"e{i}_{w0}") for i in range(6)]
                    for i in range(4):
                        nc.vector.tensor_add(et[i][:], t4[:, 2 * i, :],
                                             t4[:, 2 * i + 1, :])
                    nc.vector.tensor_add(et[4][:], et[0][:], et[1][:])
                    nc.vector.tensor_add(et[5][:], et[2][:], et[3][:])
                    ya = yaccpool.tile([128, wt], F32, tag="ya", name=f"ya_{w0}")
                    nc.vector.tensor_add(ya[:], et[4][:], et[5][:])
                    nc.vector.tensor_scalar_add(ya[:], ya[:], cpb[:, 0:1])
                    nc.sync.dma_start(
                        out=bass.AP(tensor=y_t, offset=(w0 - 16),
                                    ap=[[2048, 16], [16 * 2048, 8], [1, wt]]),
                        in_=ya[:],
                    )
    nc.compile()
    return nc


_NC_CACHE = None


def _get_nc():
    global _NC_CACHE
    if _NC_CACHE is None:
        _NC_CACHE = _build()
    return _NC_CACHE


def _host_prep(inputs):
    import ml_dtypes

    conv_w = np.asarray(inputs["conv_w"], dtype=np.float32)
    conv_b = np.asarray(inputs["conv_b"], dtype=np.float32)
    conv_gate = np.asarray(inputs["conv_gate"], dtype=np.float32)
    conv_scale = np.asarray(inputs["conv_scale"], dtype=np.float32)
    bn_gamma = np.asarray(inputs["bn_gamma"], dtype=np.float32)
    bn_beta = np.asarray(inputs["bn_beta"], dtype=np.float32)
    ct_w = np.asarray(inputs["ct_w"], dtype=np.float32)
    ct_b = np.asarray(inputs["ct_b"], dtype=np.float32)
    ct_gate = np.asarray(inputs["ct_gate"], dtype=np.float32)
    ct_scale = np.asarray(inputs["ct_scale"], dtype=np.float32)

    W1 = conv_w[:, 0, :] * (conv_gate[:, 0, :] + 1.0) * 0.5  # [c, j]
    W1 = W1 * conv_scale[:, None]
    bias1 = conv_scale * conv_b
    # w1t[j0, h, c] = W1[c, j0 + 128h]
    w1t = np.ascontiguousarray(
        W1.T.reshape(2, 128, K).transpose(1, 0, 2)).astype(ml_dtypes.bfloat16)

    vecs = np.stack([bias1, bn_gamma, bn_beta], axis=1)  # [256, 3]
    vecs = np.ascontiguousarray(vecs.reshape(2, 128, 3).transpose(1, 0, 2))

    W2 = ct_w[:, 0, :] * (ct_gate[:, 0, :] + 1.0) * 0.5  # [k, j]
    W2 = W2 * float(ct_scale[0])
    w2 = np.ascontiguousarray(W2.reshape(2, 128, K).transpose(1, 0, 2))
    w2fold = W2.reshape(K, 16, 16).sum(axis=1)  # [k, p]
    w2fold = np.ascontiguousarray(w2fold.reshape(2, 128, 16).transpose(1, 0, 2))
    cb16 = np.full(16, float(ct_scale[0]) * float(ct_b[0]), dtype=np.float32)

    return {
        "w1t": w1t,
        "vecs": vecs.astype(np.float32),
        "w2": w2.astype(np.float32),
        "w2fold": w2fold.astype(np.float32),
        "cb16": cb16,
    }


def kernel(**inputs) -> np.ndarray:
    import ml_dtypes

    x = np.asarray(inputs["x"], dtype=np.float32)  # [64, 1, 32768]
    shared = _host_prep(inputs)
    nc = _get_nc()

    in_maps = []
    for c in range(N_CORES):
        shard = x[BPC * c:BPC * (c + 1), 0, :]  # [8, T]
        xpad = np.zeros((BPC, XP), dtype=np.float32)
        xpad[:, K:K + T] = shard
        # phase layout: x_ph[b, p, n] = x_pad[b, 16n + p]
        xph = np.ascontiguousarray(
            xpad.reshape(BPC, PW, 16).transpose(0, 2, 1)).astype(ml_dtypes.bfloat16)
        m = dict(shared)
        m["x_ph"] = xph
        in_maps.append(m)

    res = run_bass_kernel_spmd(nc, in_maps, core_ids=list(range(N_CORES)))
    outs = []
    for c in range(N_CORES):
        yph = res.results[c]["y"].reshape(BPC, 16, 2048)  # [b, p, u]
        outs.append(yph.transpose(0, 2, 1).reshape(BPC, 1, T))
    return np.concatenate(outs, axis=0).astype(np.float32)


# revision 5
# speedup vs baseline: 2.8602x; 1.0460x over previous
"""Trainium2 Bass kernel for the BitwiseAutoencoder problem.

Pipeline (per core, data-parallel over batch: 8 of 64 batches per core):
  1. conv1d(1->256, k=256, stride=16, pad=256) as bf16 matmuls against a
     stride-replicated frame matrix R (one gather DMA per batch, resident).
  2. PSUM eviction fuses relu + bias on the Activation engine (multi-bank
     ops), writing H directly as bf16; the eviction's accum_out produces
     sum(h) for free.  sum(h^2) comes from DVE bn_stats on most units plus
     an ACT Square-with-accum on the trailing group of each half.
  3. [Sh, Sh2] all-gathered across the 8 cores; BN affine folded into the
     transposed-conv weights (a*W2, bf16) and a per-phase bias vector.
  4. convT(256->1, k=256, stride=16) as bf16 matmuls; tap-half fold done in
     PSUM via shifted rhs; the remaining 8 tap groups are regrouped through
     a DRAM bounce (one 4-dim gather DMA per tile) and folded with a bf16
     add tree on DVE.  Output is written phase-major (contiguous DMA) and
     transposed on the host.

Self-contained: shapes/sharding hardcoded for x: [64, 1, 32768] f32, 8 cores.
"""

import numpy as np

import concourse.bass as bass
from concourse import bacc, mybir, tile
from concourse.bass_utils import run_bass_kernel_spmd

N_CORES = 8
B_FULL = 64
BPC = B_FULL // N_CORES  # 8 batches per core
T = 32768
K = 256
S = 16
BN_EPS = 1e-5

XP = T + 2 * K  # padded x length per batch (33280)
L = (T + 2 * K - K) // S + 1  # conv output length (2065)
RW = 2073  # R width: l in [0, 2064+8]
PW = XP // S  # 2080 phase columns

UW = 413  # conv matmul unit width (L = 5*413)
# per-cc eviction groups in units: small leading groups cut pipeline lead-in
GROUP_UNITS = [1, 3] + [4] * 9
G_STARTS = [0, 1, 4, 8, 12, 16, 20, 24, 28, 32, 36]
NGC = len(GROUP_UNITS)  # 11 per cc

# deconv output tiles over u' in [16, 2064); last tile split for a short drain
U_TILES = [(16, 683), (699, 683), (1382, 341), (1723, 341)]
OFW = 690  # of2 free width (max wt + 7)

F32 = mybir.dt.float32
BF16 = mybir.dt.bfloat16
AF = mybir.ActivationFunctionType
ALU = mybir.AluOpType


def _flat_ap(tl, n0, dims):
    """Raw AP over an SBUF tile at flat free-offset n0 with given free dims."""
    full = tl[:]
    return bass.AP(tensor=full.tensor, offset=full.offset + n0,
                   ap=[[full.ap[0][0], 128]] + dims)


def _build():
    nc = bacc.Bacc("TRN2", target_bir_lowering=False, debug=False)

    # ---- external I/O ----
    xph_t = nc.dram_tensor("x_ph", [BPC, 16, PW], BF16, kind="ExternalInput")
    w1t_t = nc.dram_tensor("w1t", [128, 2, K], BF16, kind="ExternalInput")
    vecs_t = nc.dram_tensor("vecs", [128, 2, 3], F32, kind="ExternalInput")
    w2_t = nc.dram_tensor("w2", [128, 2, K], F32, kind="ExternalInput")
    w2fold_t = nc.dram_tensor("w2fold", [128, 2, 16], F32, kind="ExternalInput")
    cb16_t = nc.dram_tensor("cb16", [16], F32, kind="ExternalInput")
    y_t = nc.dram_tensor("y", [BPC, 16, 2048], F32, kind="ExternalOutput")

    with tile.TileContext(nc) as tc:
        with (
            tc.tile_pool(name="persist", bufs=1) as persist,
            tc.tile_pool(name="sqpool", bufs=1) as sqpool,
            tc.tile_pool(name="of2pool", bufs=2) as of2pool,
            tc.tile_pool(name="t4pool", bufs=2) as t4pool,
            tc.tile_pool(name="etpool", bufs=2) as etpool,
            tc.tile_pool(name="yacc", bufs=2) as yaccpool,
            tc.tile_pool(name="smalls", bufs=1) as smalls,
            tc.tile_pool(name="dram", bufs=1, space="DRAM") as dram,
        ):
            # R frame matrices, one per batch, all resident.  Even batches go
            # through SWDGE (Pool) so the first loads overlap the HWDGE queue.
            R = [persist.tile([128, RW], BF16, tag=f"R{b}", name=f"R{b}")
                 for b in range(BPC)]

            def load_r(b):
                eng = nc.gpsimd if b % 2 == 0 else nc.sync
                eng.dma_start(
                    out=R[b][:],
                    in_=bass.AP(tensor=xph_t, offset=b * XP,
                                ap=[[1, 8], [PW, 16], [1, RW]]),
                )

            load_r(0)
            w1t_sb = persist.tile([128, 2, K], BF16, tag="w1t")
            nc.sync.dma_start(out=w1t_sb[:], in_=w1t_t[:, :, :])
            vecs_sb = persist.tile([128, 2, 3], F32, tag="vecs")
            nc.sync.dma_start(out=vecs_sb[:], in_=vecs_t[:, :, :])
            for b in (2, 1, 4, 3, 6, 5):
                load_r(b)
            w2_sb = persist.tile([128, 2, K], F32, tag="w2")
            nc.sync.dma_start(out=w2_sb[:], in_=w2_t[:, :, :])
            load_r(7)
            w2fold_sb = persist.tile([128, 2, 16], F32, tag="w2fold")
            nc.sync.dma_start(out=w2fold_sb[:], in_=w2fold_t[:, :, :])
            cb_sb = persist.tile([16, 1], F32, tag="cb")
            nc.sync.dma_start(out=cb_sb[:], in_=cb16_t[:])
            eps_sb = persist.tile([128, 1], F32, tag="eps")
            nc.vector.memset(eps_sb[:], BN_EPS)
            junk = smalls.tile([128, 1], F32, tag="junk")
            # preload the Relu/Square activation table set while R loads run
            nc.scalar.activation(out=junk[:], in_=eps_sb[:], func=AF.Relu)

            # H: conv output (post-relu) bf16, flat layout (cc, b, l)
            H = persist.tile([128, 2, BPC, L], BF16, tag="H", name="H")
            sums1 = persist.tile([128, 2, NGC], F32, tag="s1", name="s1")
            sums2a = persist.tile([128, 2], F32, tag="s2a", name="s2a")
            stats = persist.tile([128, 80, 6], F32, tag="st", name="st")
            sq = sqpool.tile([128, 4 * UW], BF16, tag="sq", name="sq")

            # ================= phase 1: conv + stats =================
            with tc.tile_pool(name="psum_conv", bufs=2, space="PSUM") as pconv:
                for cc in range(2):
                    for gq in range(NGC):
                        nu_g = GROUP_UNITS[gq]
                        u0 = G_STARTS[gq]
                        ps = pconv.tile([128, 4, 512], F32, tag="pc")
                        for i in range(nu_g):
                            w = u0 + i  # within-cc unit: 5*b + gi
                            b, gi = w // 5, w % 5
                            l0 = UW * gi
                            for h in range(2):
                                nc.tensor.matmul(
                                    ps[:, i, 0:UW],
                                    w1t_sb[:, h, 128 * cc:128 * (cc + 1)],
                                    R[b][:, l0 + 8 * h:l0 + 8 * h + UW],
                                    start=(h == 0), stop=(h == 1),
                                )
                        n0 = UW * u0 + 16520 * cc
                        out_ap = _flat_ap(H, n0, [[UW, nu_g], [1, UW]])
                        # relu+bias eviction -> bf16 H; accum gives sum(h)
                        nc.scalar.activation(
                            out=out_ap, in_=ps[:, 0:nu_g, 0:UW], func=AF.Relu,
                            bias=vecs_sb[:, cc, 0:1], scale=1.0,
                            accum_out=sums1[:, cc, gq:gq + 1],
                        )
                        if gq == NGC - 1:
                            # trailing group's sum(h^2) on ACT
                            nc.scalar.activation(
                                out=sq[:, 0:nu_g * UW],
                                in_=_flat_ap(H, n0, [[1, nu_g * UW]]),
                                func=AF.Square,
                                accum_out=sums2a[:, cc:cc + 1],
                            )
                        else:
                            for i in range(nu_g):
                                u = 40 * cc + u0 + i
                                nc.vector.bn_stats(
                                    out=stats[:, u, :],
                                    in_=_flat_ap(H, 413 * (u0 + i) + 16520 * cc,
                                                 [[1, UW]]),
                                )
            # preload the Sqrt/Copy table set during the collective window
            nc.scalar.activation(out=junk[:], in_=eps_sb[:], func=AF.Sqrt,
                                 bias=eps_sb[:, 0:1])

            # ================= phase 2: global BN =================
            bounce_in = dram.tile([128, 4], F32)
            bounce_out = dram.tile([N_CORES, 128, 4], F32)
            pk = smalls.tile([128, 4], F32, tag="pk")
            n_rest = float(36 * UW)
            for cc in range(2):
                # total sum(h) for this half
                nc.vector.reduce_sum(pk[:, 2 * cc:2 * cc + 1],
                                     sums1[:, cc, :],
                                     axis=mybir.AxisListType.X)
                # sum(h^2): ACT trailing group + bn_stats units (first 36)
                mv = smalls.tile([128, 2], F32, tag=f"mv{cc}", name=f"mv{cc}")
                nc.vector.bn_aggr(out=mv[:],
                                  in_=stats[:, 40 * cc:40 * cc + 36, :])
                e2r = smalls.tile([128, 1], F32, tag=f"e2r{cc}", name=f"e2r{cc}")
                nc.vector.tensor_mul(e2r[:], mv[:, 0:1], mv[:, 0:1])
                nc.vector.tensor_add(e2r[:], e2r[:], mv[:, 1:2])
                nc.vector.tensor_scalar_mul(e2r[:], e2r[:], n_rest)
                nc.vector.tensor_add(pk[:, 2 * cc + 1:2 * cc + 2],
                                     e2r[:], sums2a[:, cc:cc + 1])
            nc.sync.dma_start(out=bounce_in[:, :], in_=pk[:])
            nc.gpsimd.collective_compute(
                "AllGather",
                mybir.AluOpType.bypass,
                replica_groups=[list(range(N_CORES))],
                ins=[bounce_in.opt()],
                outs=[bounce_out.opt()],
            )
            gall = smalls.tile([128, 4, N_CORES], F32, tag="gall")
            nc.sync.dma_start(
                out=gall[:],
                in_=bass.AP(tensor=bounce_out.tensor, offset=bounce_out.offset,
                            ap=[[4, 128], [1, 4], [512, N_CORES]]),
            )
            gsum = smalls.tile([128, 4], F32, tag="gsum")
            nc.vector.reduce_sum(gsum[:], gall[:], axis=mybir.AxisListType.X)
            inv_n = 1.0 / (N_CORES * BPC * L)
            # fold BN scale into deconv weights -> bf16 (kc = 0 first so the
            # first deconv matmuls can start as early as possible)
            w2a = persist.tile([128, 2, K], BF16, tag="w2a", name="w2a")
            a_sb, d_sb = [], []
            for cc in range(2):
                mE = smalls.tile([128, 2], F32, tag=f"mE{cc}", name=f"mE{cc}")
                nc.vector.tensor_scalar_mul(mE[:], gsum[:, 2 * cc:2 * cc + 2], inv_n)
                gvar = smalls.tile([128, 1], F32, tag=f"gv{cc}", name=f"gv{cc}")
                nc.vector.tensor_mul(gvar[:], mE[:, 0:1], mE[:, 0:1])
                nc.vector.tensor_sub(gvar[:], mE[:, 1:2], gvar[:])
                sd = smalls.tile([128, 1], F32, tag=f"sd{cc}", name=f"sd{cc}")
                nc.scalar.activation(out=sd[:], in_=gvar[:], func=AF.Sqrt,
                                     bias=eps_sb[:, 0:1], scale=1.0)
                rinv = smalls.tile([128, 1], F32, tag=f"ri{cc}", name=f"ri{cc}")
                nc.vector.reciprocal(rinv[:], sd[:])
                a = smalls.tile([128, 1], F32, tag=f"a{cc}", name=f"a{cc}")
                nc.vector.tensor_mul(a[:], rinv[:], vecs_sb[:, cc, 1:2])
                nc.vector.tensor_scalar_mul(w2_sb[:, cc, :], w2_sb[:, cc, :],
                                            a[:, 0:1])
                nc.vector.tensor_copy(w2a[:, cc, :], w2_sb[:, cc, :])
                d = smalls.tile([128, 1], F32, tag=f"d{cc}", name=f"d{cc}")
                nc.vector.tensor_mul(d[:], a[:], mE[:, 0:1])
                nc.vector.tensor_sub(d[:], vecs_sb[:, cc, 2:3], d[:])
                a_sb.append(a)
                d_sb.append(d)

            with (
                tc.tile_pool(name="psum_cp", bufs=1, space="PSUM") as psum_cp,
                tc.tile_pool(name="psum_dec", bufs=4, space="PSUM") as pdec,
            ):
                # per-phase bias: cp[p] = sum_k w2fold[k, p] d[k] + ct_scale*ct_b
                pcp = psum_cp.tile([16, 1], F32, tag="pcp")
                nc.tensor.matmul(pcp[:], w2fold_sb[:, 0, :], d_sb[0][:],
                                 start=True, stop=False)
                nc.tensor.matmul(pcp[:], w2fold_sb[:, 1, :], d_sb[1][:],
                                 start=False, stop=True)
                cp16 = smalls.tile([16, 1], F32, tag="cp16")
                nc.vector.tensor_add(cp16[:], pcp[:], cb_sb[:])
                cp_dram = dram.tile([16], F32)
                nc.sync.dma_start(out=cp_dram[:], in_=cp16[:])
                # cpb[8p + b] = cp[p]
                cpb = smalls.tile([128, 1], F32, tag="cpb")
                nc.sync.dma_start(
                    out=cpb[:],
                    in_=bass.AP(tensor=cp_dram.tensor, offset=cp_dram.offset,
                                ap=[[1, 16], [0, 8], [0, 1]]),
                )

                # ================= phase 3: deconv =================
                of2d = [dram.tile([128, BPC, OFW], BF16, name=f"of2d{i}")
                        for i in range(2)]
                for ti, (w0, wt) in enumerate(U_TILES):
                    w7 = wt + 7
                    dbuf = of2d[ti % 2]
                    for b in range(BPC):
                        if b % 2 == 0:
                            of2 = of2pool.tile([128, 2, OFW], BF16, tag="OF2",
                                               name=f"of2_{w0}_{b}")
                        # tap-half fold in PSUM: OF2[r, n] = OF[r, n] + OF[r+128, n-8]
                        for si, s0 in enumerate(range(0, w7, 504)):
                            sw = min(504, w7 - s0)
                            ps = pdec.tile([128, 504], F32, tag="pd")
                            nmm = 0
                            for kc in range(2):
                                for th, off in ((0, 7), (128, 15)):
                                    nc.tensor.matmul(
                                        ps[:, :sw],
                                        w2a[:, kc, th:th + 128],
                                        H[:, kc, b, w0 - off + s0:w0 - off + s0 + sw],
                                        start=(nmm == 0), stop=(nmm == 3),
                                    )
                                    nmm += 1
                            # evictions split between ACT and DVE
                            dst = of2[:, b % 2, s0:s0 + sw]
                            if si == 0 and wt > 341:
                                nc.scalar.copy(dst, ps[:, :sw])
                            elif wt > 341:
                                nc.vector.tensor_copy(dst, ps[:, :sw])
                            elif b % 2 == 0:
                                nc.scalar.copy(dst, ps[:, :sw])
                            else:
                                nc.vector.tensor_copy(dst, ps[:, :sw])
                        if b % 2 == 1:
                            # bounce two batches of taps to DRAM (SWDGE)
                            nc.gpsimd.dma_start(
                                out=dbuf[:, b - 1:b + 1, 0:w7],
                                in_=of2[:, :, 0:w7])
                    # partition regroup via one 4-dim gather:
                    # t4[8p + b, m, u] = of2[16m + p, b, u + 7 - m]
                    t4 = t4pool.tile([128, 8, 683], BF16, tag="T4",
                                     name=f"t4_{w0}")
                    nc.sync.dma_start(
                        out=t4[:, :, 0:wt],
                        in_=bass.AP(
                            tensor=dbuf.tensor, offset=dbuf.offset + 7,
                            ap=[[BPC * OFW, 16], [OFW, 8],
                                [16 * BPC * OFW - 1, 8], [1, wt]]),
                    )
                    # fold the 8 tap groups: bf16 add tree, f32 root
                    et = [etpool.tile([128, 683], BF16, tag=f"e{i}",
                                      name=f"e{i}_{w0}") for i in range(6)]
                    for i in range(4):
                        nc.vector.tensor_add(et[i][:, :wt], t4[:, 2 * i, :wt],
                                             t4[:, 2 * i + 1, :wt])
                    nc.vector.tensor_add(et[4][:, :wt], et[0][:, :wt], et[1][:, :wt])
                    nc.vector.tensor_add(et[5][:, :wt], et[2][:, :wt], et[3][:, :wt])
                    ya = yaccpool.tile([128, 683], F32, tag="ya", name=f"ya_{w0}")
                    nc.vector.tensor_add(ya[:, :wt], et[4][:, :wt], et[5][:, :wt])
                    nc.vector.tensor_scalar_add(ya[:, :wt], ya[:, :wt], cpb[:, 0:1])
                    nc.sync.dma_start(
                        out=bass.AP(tensor=y_t, offset=(w0 - 16),
                                    ap=[[2048, 16], [16 * 2048, 8], [1, wt]]),
                        in_=ya[:, :wt],
                    )
    nc.compile()
    return nc


_NC_CACHE = None


def _get_nc():
    global _NC_CACHE
    if _NC_CACHE is None:
        _NC_CACHE = _build()
    return _NC_CACHE


def _host_prep(inputs):
    import ml_dtypes

    conv_w = np.asarray(inputs["conv_w"], dtype=np.float32)
    conv_b = np.asarray(inputs["conv_b"], dtype=np.float32)
    conv_gate = np.asarray(inputs["conv_gate"], dtype=np.float32)
    conv_scale = np.asarray(inputs["conv_scale"], dtype=np.float32)
    bn_gamma = np.asarray(inputs["bn_gamma"], dtype=np.float32)
    bn_beta = np.asarray(inputs["bn_beta"], dtype=np.float32)
    ct_w = np.asarray(inputs["ct_w"], dtype=np.float32)
    ct_b = np.asarray(inputs["ct_b"], dtype=np.float32)
    ct_gate = np.asarray(inputs["ct_gate"], dtype=np.float32)
    ct_scale = np.asarray(inputs["ct_scale"], dtype=np.float32)

    W1 = conv_w[:, 0, :] * (conv_gate[:, 0, :] + 1.0) * 0.5  # [c, j]
    W1 = W1 * conv_scale[:, None]
    bias1 = conv_scale * conv_b
    # w1t[j0, h, c] = W1[c, j0 + 128h]
    w1t = np.ascontiguousarray(
        W1.T.reshape(2, 128, K).transpose(1, 0, 2)).astype(ml_dtypes.bfloat16)

    vecs = np.stack([bias1, bn_gamma, bn_beta], axis=1)  # [256, 3]
    vecs = np.ascontiguousarray(vecs.reshape(2, 128, 3).transpose(1, 0, 2))

    W2 = ct_w[:, 0, :] * (ct_gate[:, 0, :] + 1.0) * 0.5  # [k, j]
    W2 = W2 * float(ct_scale[0])
    w2 = np.ascontiguousarray(W2.reshape(2, 128, K).transpose(1, 0, 2))
    w2fold = W2.reshape(K, 16, 16).sum(axis=1)  # [k, p]
    w2fold = np.ascontiguousarray(w2fold.reshape(2, 128, 16).transpose(1, 0, 2))
    cb16 = np.full(16, float(ct_scale[0]) * float(ct_b[0]), dtype=np.float32)

    return {
        "w1t": w1t,
        "vecs": vecs.astype(np.float32),
        "w2": w2.astype(np.float32),
        "w2fold": w2fold.astype(np.float32),
        "cb16": cb16,
    }


def kernel(**inputs) -> np.ndarray:
    import ml_dtypes

    x = np.asarray(inputs["x"], dtype=np.float32)  # [64, 1, 32768]
    shared = _host_prep(inputs)
    nc = _get_nc()

    in_maps = []
    for c in range(N_CORES):
        shard = x[BPC * c:BPC * (c + 1), 0, :]  # [8, T]
        xpad = np.zeros((BPC, XP), dtype=np.float32)
        xpad[:, K:K + T] = shard
        # phase layout: x_ph[b, p, n] = x_pad[b, 16n + p]
        xph = np.ascontiguousarray(
            xpad.reshape(BPC, PW, 16).transpose(0, 2, 1)).astype(ml_dtypes.bfloat16)
        m = dict(shared)
        m["x_ph"] = xph
        in_maps.append(m)

    res = run_bass_kernel_spmd(nc, in_maps, core_ids=list(range(N_CORES)))
    outs = []
    for c in range(N_CORES):
        yph = res.results[c]["y"].reshape(BPC, 16, 2048)  # [b, p, u]
        outs.append(yph.transpose(0, 2, 1).reshape(BPC, 1, T))
    return np.concatenate(outs, axis=0).astype(np.float32)
